# revision 45
# baseline (speedup 1.0000x reference)
"""Trainium2 Bass kernel for BaseLIDIA weighted overlap-add (fold) network.

Math (derived from the reference):
  out[t,ch,y,x] = 0.5 * img[t,ch,y,x] / cnt[t,y,x] + mean(noisy[t,ch])
  img[ch,y,x]   = sum_{i,j in 0..4} deno[t, (y+4-i)*536 + (x+4-j), ch*25+i*5+j]
                                    * w[t, (y+4-i)*536 + (x+4-j)]
  cnt[y,x]      = sum_{i,j in 0..4} w[t, (y+4-i)*536 + (x+4-j)]
(`inds` is unused by the reference; the pre/post scaling collapses so that the
only use of `noisy` is its raw per-channel mean.)

Sharding: 8 cores = 2 frames x 4 row-bands of 133 output rows. Each core gets
patch rows [133b, 133b+137) (4-row halo) of its frame.

Per-core on-device algorithm (patch columns q on SBUF partitions, host layout
[q, d'=(j,i,ch), r] with r padded to 138 so every engine AP is unit-stride
innermost; d reordered on the host so each j-group of taps is contiguous):
  - deno band slab [q<=128, 75, 138] bf16 loads in 2 chunk DMAs per x-block
    on the SP HWDGE ring (w rides the same ring, prefetched a pass ahead —
    on the ACT ring its issue order serialized every pass boundary).
  - wd = deno * w in 5 j-chunk DVE tensor_tensors (w broadcast over d as the
    OUTER free dim; unit-stride inner -> 2x_1P packed mode).  Each chunk
    immediately feeds its 5 img taps so the PE never idles long enough for
    the HAM clock gate to re-throttle it to 1.2 GHz.
  - img[x, (ch,y)] = PSUM accumulation of 25 shifted matmuls (one per fold
    tap (i,j)): stationary 0/1 shift matrix (padded to 128 cols so FWL
    engages) handles x+4-j, the rhs AP offset (4-i) handles y+4-i.
  - cnt for all 5 blocks batches at the top of each pass: banded 2.0-matrix
    lhsT folds the j-sum (5 matmuls per PSUM tile, block dim on the rhs free
    AP); folds the final *0.5 into 1/(2 cnt).  Per-block DVE reciprocals
    trail one block behind so they never stall the multiply FIFO.
  - finals: ACT evacuates img PSUM -> one DVE bf16 2x multiply by 1/(2cnt)
    -> 3 ACT Identity activations add the per-channel mean (per-partition
    bias AP).  GpSimd is kept idle: its SBUF port is shared with the DVE
    and streaming on it slows the multiply TTs ~60%.
  - the whole pass's output stages in one [124, 5*399] tile and ships as a
    single 124-descriptor DMA (many sub-1KB descriptors double the
    semaphore-packet load on SDMA engines 0-3, the DMA bottleneck).
  - the 40-partition tail x-block sits at partition base 64 so its
    descriptors land on the underloaded odd SDMA engines.
Per-rep steady state is HBM-bound: ~12 MB/core/rep at the ~330 GB/s
effective per-core bandwidth when all 8 cores stream (plus ~6us/rep of
semaphore/refill packets pinned to SDMA engine 0).
"""

import ml_dtypes
import numpy as np

import concourse.bass as bass
import concourse.mybir as mybir
import concourse.tile as tile
from concourse import bacc
from concourse.bass_utils import run_bass_kernel_spmd

F32 = mybir.dt.float32
BF16 = mybir.dt.bfloat16
AX = mybir.AxisListType
ALU = mybir.AluOpType
ACTF = mybir.ActivationFunctionType

PS = 5
PH = PW = 536
H = W = 532
PD = 75
NBAND = 4
BAND_Y = 133          # output rows per band
BAND_R = 137          # patch rows per band (halo of PS-1)
RP = 138              # padded patch-row pitch (even -> keeps bf16 2x packing)
NPIX_CH = H * W       # 283024, per-channel pixel count
FD = 3 * BAND_Y       # 399 free elements of the img/out tiles

# x-blocks: (x0, nx, nq, pb)  with q-range [x0, x0 + nq), loaded at SBUF
# partition base pb.  The 40-partition tail block sits at pb=64: partitions
# 0-39 map to the even SDMA engines, which already carry the fullest load;
# 64-103 map to the underloaded odd engines, leveling the DMA stream.
XBLKS = [(0, 124, 128, 0), (124, 124, 128, 0), (248, 124, 128, 0),
         (372, 124, 128, 0), (496, 36, 40, 64)]


def _ap_p(base: bass.AP, npart: int, extra_off: int, dims):
    """Custom strided view of a tile: partition dim of `base` overridden to
    `npart`, free dims replaced."""
    part = [[base.ap[0][0], npart]]
    return bass.AP(base.tensor, base.offset + extra_off, part + [list(d) for d in dims])


def build_program(reps: int = 1, ablate: str = ""):
    """Build (and compile) the single-core Bass program. SPMD: all 8 cores run
    it on their own band slice. Returns the Bacc object."""
    nc = bacc.Bacc("TRN2", target_bir_lowering=False, debug=False,
                   enable_asserts=False, num_devices=8)

    deno_d = nc.dram_tensor("deno", [PW, PD, RP], BF16, kind="ExternalInput")
    wt_d = nc.dram_tensor("wt", [128, len(XBLKS) * RP], BF16,
                          kind="ExternalInput")
    noisy_d = nc.dram_tensor("noisy", [3, H, W], BF16, kind="ExternalInput")
    # out layout [x_local=124, (block, ch, y)]: one contiguous 4KB run per
    # partition -> the whole pass's output ships as ONE 124-descriptor DMA
    # (5 per-block DMAs = 620 sub-1KB descriptors cost ~2x the semaphore
    # packets on SDMA engines 0-3, the busiest ones).
    out_d = nc.dram_tensor("out", [124, len(XBLKS) * FD], BF16,
                           kind="ExternalOutput")

    with tile.TileContext(nc) as tc:
        with (
            tc.tile_pool(name="const", bufs=1) as const_p,
            tc.tile_pool(name="deno", bufs=7) as deno_p,
            tc.tile_pool(name="wq", bufs=2) as wq_p,
            tc.tile_pool(name="small", bufs=2) as small_p,
            tc.tile_pool(name="o1", bufs=2) as o1_p,
            tc.tile_pool(name="stage", bufs=3) as stage_p,
            tc.tile_pool(name="noisy", bufs=1) as noisy_p,
            tc.tile_pool(name="psI", bufs=3, space=bass.MemorySpace.PSUM) as psI,
            tc.tile_pool(name="psC", bufs=2, space=bass.MemorySpace.PSUM) as psC,
            tc.tile_pool(name="psW", bufs=1, space=bass.MemorySpace.PSUM) as psW,
        ):
            # ---- constants ----
            # shift identities, padded to 128 cols so FWL engages:
            # shifts[pb][j][q, m] = 1.0 iff q - pb == m + 4 - j
            def mkshift(tag, j, v, pb):
                sh = const_p.tile([128, 128], BF16, tag=tag)
                nc.gpsimd.memset(sh[:], 0.0)
                nc.gpsimd.affine_select(
                    out=sh[:], in_=sh[:], compare_op=ALU.not_equal, fill=v,
                    base=j - 4 - pb, pattern=[[-1, 128]],
                    channel_multiplier=1)
                return sh
            # banded cnt matrix: band2[pb][q, m] = 2.0 iff 0 <= q-pb-m <= 4
            # (sum over j of the 5 shift matrices, scaled by 2)
            def mkband(tag, pb):
                bd = const_p.tile([128, 128], BF16, tag=tag)
                nc.gpsimd.memset(bd[:], 0.0)
                for j in range(PS):
                    nc.gpsimd.affine_select(
                        out=bd[:], in_=bd[:], compare_op=ALU.not_equal,
                        fill=2.0, base=j - 4 - pb, pattern=[[-1, 128]],
                        channel_multiplier=1)
                return bd
            PBS = sorted({pb for (_, _, _, pb) in XBLKS})
            shifts = {pb: [mkshift(f"shift{pb}_{j}", j, 1.0, pb)
                           for j in range(PS)] for pb in PBS}
            band2 = {pb: mkband(f"band2_{pb}", pb) for pb in PBS}

            ones76 = const_p.tile([76, 1], BF16, tag="ones76")
            nc.gpsimd.memset(ones76[:], 1.0)
            onesrow = const_p.tile([1, 128], F32, tag="onesrow")
            nc.gpsimd.memset(onesrow[:], 1.0 / NPIX_CH)

            # ---- per-channel means of raw noisy ----
            sums = const_p.tile([1, 3], F32, tag="sums")
            for ch in range(3):
                npix = noisy_p.tile([76, 3724], BF16, tag="noisy")
                nc.sync.dma_start(
                    out=npix[:],
                    in_=bass.AP(noisy_d, ch * NPIX_CH, [[3724, 76], [1, 3724]]))
                msum = psW.tile([1, 512], F32, tag="psw")
                nchunk = (3724 + 511) // 512
                for ci in range(nchunk):
                    c0 = ci * 512
                    n = min(512, 3724 - c0)
                    nc.tensor.matmul(
                        out=msum[0:1, 0:n],
                        lhsT=ones76[:],
                        rhs=npix[:, c0:c0 + n],
                        start=(ci == 0), stop=(ci == nchunk - 1))
                nc.vector.tensor_reduce(
                    out=sums[0:1, ch:ch + 1], in_=msum[0:1, 0:512],
                    axis=AX.X, op=ALU.add)
            mrep_ps = psW.tile([128, 3], F32, tag="psw")
            nc.tensor.matmul(out=mrep_ps[:], lhsT=onesrow[:],
                             rhs=sums[:], start=True, stop=True)
            mean_rep = const_p.tile([128, 3], F32, tag="mean_rep")
            nc.scalar.copy(mean_rep[:], mrep_ps[:])


            # ---- main loop ----
            # reps>1 wraps the body in a For_i hardware loop (for timing runs)
            UNROLL = 48
            import contextlib
            loop_cm = (tc.For_i(0, (reps + UNROLL - 1) // UNROLL, 1,
                                staggered_reset=True)
                       if reps > 1 else contextlib.nullcontext())
            n_passes = UNROLL if reps > 1 else 1
            if "nomm" in ablate and "nofin" not in ablate:
                ablate = ablate + " nofin"
            with loop_cm:
              # finals are software-pipelined one block behind the front-end
              # so the PE-consuming ops never stall their engine queues
              # waiting on this block's matmuls.
              pend = []
              rc_pend = []
              for _pass in range(n_passes):
                # wq rides the SP HWDGE ring: on the ACT ring its issue sits
                # behind the previous pass's PSUM-evac COPY + out-DMA (which
                # wait on that pass's tail), serializing every pass boundary
                # by ~15us.  On SP it issues as soon as the previous slab
                # D2Ds have, draining during the previous pass's compute.
                wq = wq_p.tile([128, len(XBLKS) * RP], BF16, tag="wq")
                nc.sync.dma_start(out=wq[:], in_=wt_d[:, :])

                # cnt for ALL blocks (depends only on wq): blocks 0-2 in one
                # PSUM tile, 3-4 in another.  Banded lhsT folds the j-sum;
                # the i-shift is the rhs AP offset; the block dim rides the
                # rhs free AP (the band matrix is block-local in q).  Emitted
                # as a closure so the matmuls land in the PE FIFO after block
                # 0's img matmuls (never delaying them), while the per-block
                # reciprocals trail one block behind in the DVE FIFO.
                cntA = psC.tile([128, 3 * BAND_Y], F32, tag="cntA")
                cntB = psC.tile([128, 2 * BAND_Y], F32, tag="cntB")
                rcA = small_p.tile([124, 3 * BAND_Y], F32, tag="rcA")
                rcB = small_p.tile([124, 2 * BAND_Y], F32, tag="rcB")
                # bf16 copies (pitch 134 keeps per-block slices 4B-aligned
                # for the finals' DVE 2x packed mode)
                rcAb = small_p.tile([124, 3 * 134], BF16, tag="rcAb")
                rcBb = small_p.tile([124, 2 * 134], BF16, tag="rcBb")
                # whole-pass output staging tile (one 124-descriptor DMA at
                # pass end).  The block-4 tail rows are never computed; zero
                # them so the (ignored-by-host) DMA bytes are finite.
                st_all = stage_p.tile([124, len(XBLKS) * FD], BF16, tag="st")
                nc.gpsimd.memset(st_all[0:124, 4 * FD:5 * FD], 0.0)

                def cnt_mms(cntA=cntA, cntB=cntB):
                    WQS = len(XBLKS) * RP
                    for i in range(PS):
                        nc.tensor.matmul(
                            out=cntA[:, :],
                            lhsT=band2[0][:, :],
                            rhs=_ap_p(wq[:], 128, (4 - i),
                                      [[RP, 3], [1, BAND_Y]]),
                            start=(i == 0), stop=(i == PS - 1))
                    # block 3 (pb=0) opens the cntB bank; block 4 (pb=64)
                    # writes the disjoint second slice (has_written=0 there
                    # after the group start -> first write overwrites).
                    for i in range(PS):
                        nc.tensor.matmul(
                            out=cntB[:, 0:BAND_Y],
                            lhsT=band2[0][:, :],
                            rhs=_ap_p(wq[:], 128, 3 * RP + (4 - i),
                                      [[1, BAND_Y]]),
                            start=(i == 0), stop=False)
                    x0, nx, nq, pb = XBLKS[4]
                    for i in range(PS):
                        nc.tensor.matmul(
                            out=cntB[:, BAND_Y:2 * BAND_Y],
                            lhsT=band2[pb][pb:pb + nq, :],
                            rhs=_ap_p(wq[:], nq,
                                      pb * WQS + 4 * RP + (4 - i),
                                      [[1, BAND_Y]]),
                            start=False, stop=(i == PS - 1))

                def mk_rc(b, cntA=cntA, cntB=cntB, rcA=rcA, rcB=rcB,
                          rcAb=rcAb, rcBb=rcBb):
                    # per-block reciprocal slice (block 4 only has 36 cols;
                    # the rest of its cnt slice is 0 -> skip, 1/0 is inf),
                    # then a bf16 downconvert for the finals 2x TT
                    src, dst, dstb, off = ((cntA, rcA, rcAb, b) if b < 3 else
                                           (cntB, rcB, rcBb, b - 3))
                    np_ = 124 if b < 4 else 36
                    def rc():
                        nc.vector.reciprocal_approx_fast(
                            dst[0:np_, off * BAND_Y:(off + 1) * BAND_Y],
                            src[0:np_, off * BAND_Y:(off + 1) * BAND_Y])
                        nc.vector.tensor_copy(
                            out=dstb[0:np_, off * 134:off * 134 + BAND_Y],
                            in_=dst[0:np_, off * BAND_Y:(off + 1) * BAND_Y])
                    return rc

                for b, (x0, nx, nq, pb) in enumerate(XBLKS):
                    WQS = len(XBLKS) * RP
                    DTS = PD * RP
                    dt = deno_p.tile([128, PD * RP], BF16, tag="deno")
                    # the [q, d, r] slab loads in 5 j-chunk DMAs (the host
                    # orders d as (j, i, ch), so each chunk is contiguous
                    # 4.1KB per partition): the DVE multiply for chunk j
                    # starts as soon as its fifth of the slab lands instead
                    # of waiting for the whole 20.7KB.
                    if "nodma" not in ablate:
                        # 2 DMAs per slab (j-chunks {0,1} and {2,3,4}): a
                        # compromise between early DVE start (subtile deps
                        # let TT_0 go after the first 2/5 of the slab) and
                        # DMA count — every DMA costs ~16 semaphore packets
                        # that all land on SDMA engines 0-3.
                        for (j0, j1) in ((0, 2), (2, PS)):
                            nc.sync.dma_start(
                                out=dt[pb:pb + nq,
                                       j0 * 15 * RP:j1 * 15 * RP],
                                in_=bass.AP(deno_d,
                                            x0 * PD * RP + j0 * 15 * RP,
                                            [[PD * RP, nq],
                                             [1, (j1 - j0) * 15 * RP]]))
                    img = None
                    if "nomm" not in ablate:
                        img = psI.tile([128, FD], F32, tag="img")
                    # wd = deno * w in 5 j-chunks (d-rows (j,i,ch)); each
                    # chunk immediately feeds its 5 img taps so the PE wakes
                    # every ~1.2us and the HAM clock gate stays at full rate.
                    for j in range(PS):
                        if "nott" not in ablate:
                            nc.vector.tensor_tensor(
                                out=_ap_p(dt[:], nq, pb * DTS + j * 15 * RP,
                                          [[RP, 15], [1, RP]]),
                                in0=_ap_p(dt[:], nq, pb * DTS + j * 15 * RP,
                                          [[RP, 15], [1, RP]]),
                                in1=_ap_p(wq[:], nq, pb * WQS + b * RP,
                                          [[0, 15], [1, RP]]),
                                op=ALU.mult)
                        # img[x, (ch,y)] accumulates the 5 taps of this j:
                        # tap (i,j): rhs = wd[q, d=(j,i,ch), r=y+4-i]
                        if "nomm" not in ablate:
                            for i in range(PS):
                                nc.tensor.matmul(
                                    out=img[:, :],
                                    lhsT=shifts[pb][j][pb:pb + nq, :],
                                    rhs=_ap_p(dt[:], nq,
                                              pb * DTS +
                                              (j * 15 + i * 3) * RP + (4 - i),
                                              [[RP, 3], [1, BAND_Y]]),
                                    start=(j == 0 and i == 0),
                                    stop=(j == PS - 1 and i == PS - 1))
                    if b == 0 and "nomm" not in ablate:
                        cnt_mms()

                    # finals (deferred 1 block): ACT evacuates img PSUM, one
                    # DVE bf16 2x multiply applies 1/(2cnt), then 3 ACT
                    # Identity activations add the per-channel mean (bias is
                    # a per-partition AP, so ACT can do it — keeping both
                    # the mean matmuls off the PE and the add off DVE and
                    # GpSimd, whose SBUF port contends with the DVE).
                    def finals(b=b, x0=x0, nx=nx, img=img, rcAb=rcAb,
                               rcBb=rcBb, st_all=st_all):
                        if "nofin" not in ablate:
                            rc_t, off = (rcAb, b) if b < 3 else (rcBb, b - 3)
                            o1 = o1_p.tile([124, FD], BF16, tag="o1")
                            tmp = stage_p.tile([124, FD], BF16, tag="tmp")
                            nc.scalar.copy(o1[0:nx, :], img[0:nx, :])
                            nc.vector.tensor_tensor(
                                out=tmp[0:nx, :],
                                in0=o1[0:nx, :],
                                in1=_ap_p(rc_t[:], nx, off * 134,
                                          [[0, 3], [1, BAND_Y]]),
                                op=ALU.mult)
                            for ch in range(3):
                                nc.scalar.activation(
                                    out=st_all[0:nx,
                                               b * FD + ch * BAND_Y:
                                               b * FD + (ch + 1) * BAND_Y],
                                    in_=tmp[0:nx,
                                            ch * BAND_Y:(ch + 1) * BAND_Y],
                                    func=ACTF.Identity,
                                    bias=mean_rep[0:nx, ch:ch + 1])
                        else:
                            nc.gpsimd.memset(
                                st_all[:, b * FD:(b + 1) * FD], 0.0)
                        if b == len(XBLKS) - 1 and "noout" not in ablate:
                            nc.scalar.dma_start(
                                out=bass.AP(out_d, 0,
                                            [[len(XBLKS) * FD, 124],
                                             [1, len(XBLKS) * FD]]),
                                in_=st_all[0:124, :])
                    if "nomm" not in ablate and "nofin" not in ablate:
                        rc_pend.append(mk_rc(b))
                        if len(rc_pend) > 1:
                            rc_pend.pop(0)()
                    pend.append(finals)
                    if len(pend) > 1:
                        pend.pop(0)()
              for rc in rc_pend:
                  rc()
              rc_pend.clear()
              for f in pend:
                  f()
              pend.clear()

    nc.compile()
    return nc


_CACHE = {}


def _get_program(reps: int = 1, ablate: str = ""):
    key = (reps, ablate)
    if key not in _CACHE:
        _CACHE[key] = build_program(reps, ablate)
    return _CACHE[key]


_DPERM = np.array([ch * 25 + i * 5 + j
                   for j in range(PS) for i in range(PS) for ch in range(3)])


def make_in_maps(noisy, deno, patch_weights):
    in_maps = []
    bf = ml_dtypes.bfloat16
    for core in range(8):
        t, b = divmod(core, NBAND)
        dband = deno[t].reshape(PH, PW, PD)[133 * b:133 * b + BAND_R]
        dband = dband.transpose(1, 2, 0)[:, _DPERM]  # [q=536, d'=(j,i,ch), r=137]
        dpad = np.zeros((PW, PD, RP), dtype=bf)
        dpad[:, :, :BAND_R] = dband.astype(bf)
        wband = patch_weights[t, :, 0].reshape(PH, PW)[133 * b:133 * b + BAND_R]
        wband = wband.T                            # [q=536, r=137]
        wtile = np.zeros((128, len(XBLKS) * RP), dtype=bf)
        for blk, (x0, nx, nq, pb) in enumerate(XBLKS):
            wtile[pb:pb + nq, blk * RP:blk * RP + BAND_R] = \
                wband[x0:x0 + nq].astype(bf)
        in_maps.append({
            "deno": dpad,
            "wt": wtile,
            "noisy": np.ascontiguousarray(noisy[t]).astype(bf),
        })
    return in_maps


def unpack_out(arr):
    """Device out [124, (block, ch, y)] bf16 -> [3, 133, 532] f32."""
    a = np.asarray(arr).astype(np.float32).reshape(124, len(XBLKS), 3, BAND_Y)
    x = np.empty((W, 3, BAND_Y), np.float32)
    for b, (x0, nx, nq, pb) in enumerate(XBLKS):
        x[x0:x0 + nx] = a[0:nx, b]
    return x.transpose(1, 2, 0)


def assemble(results):
    out = np.empty((2, 3, H, W), dtype=np.float32)
    for core in range(8):
        t, b = divmod(core, NBAND)
        out[t, :, 133 * b:133 * b + BAND_Y, :] = unpack_out(results[core]["out"])
    return out


def kernel(noisy, deno, patch_weights, inds=None, pixels_h=None, pixels_w=None,
           patches_h=None, patches_w=None, **_):
    noisy = np.asarray(noisy, dtype=np.float32)
    deno = np.asarray(deno, dtype=np.float32)
    patch_weights = np.asarray(patch_weights, dtype=np.float32)
    nc = _get_program()
    res = run_bass_kernel_spmd(nc, make_in_maps(noisy, deno, patch_weights),
                               core_ids=list(range(8)))
    return assemble(res.results)


# revision 48
# speedup vs baseline: 1.0144x; 1.0144x over previous
"""Trainium2 Bass kernel for BaseLIDIA weighted overlap-add (fold) network.

Math (derived from the reference):
  out[t,ch,y,x] = 0.5 * img[t,ch,y,x] / cnt[t,y,x] + mean(noisy[t,ch])
  img[ch,y,x]   = sum_{i,j in 0..4} deno[t, (y+4-i)*536 + (x+4-j), ch*25+i*5+j]
                                    * w[t, (y+4-i)*536 + (x+4-j)]
  cnt[y,x]      = sum_{i,j in 0..4} w[t, (y+4-i)*536 + (x+4-j)]
(`inds` is unused by the reference; the pre/post scaling collapses so that the
only use of `noisy` is its raw per-channel mean.)

Sharding: 8 cores = 2 frames x 4 row-bands of 133 output rows. Each core gets
patch rows [133b, 133b+137) (4-row halo) of its frame.

Per-core on-device algorithm (patch columns q on SBUF partitions, host layout
[q, d'=(j,i,ch), r] with r padded to 138 so every engine AP is unit-stride
innermost; d reordered on the host so each j-group of taps is contiguous):
  - deno band slab [q<=128, 75, 138] bf16 loads in 2 chunk DMAs per x-block
    on the SP HWDGE ring (w rides the same ring, prefetched a pass ahead —
    on the ACT ring its issue order serialized every pass boundary).
  - wd = deno * w in 5 j-chunk DVE tensor_tensors (w broadcast over d as the
    OUTER free dim; unit-stride inner -> 2x_1P packed mode).  Each chunk
    immediately feeds its 5 img taps so the PE never idles long enough for
    the HAM clock gate to re-throttle it to 1.2 GHz.
  - img[x, (ch,y)] = PSUM accumulation of 25 shifted matmuls (one per fold
    tap (i,j)): stationary 0/1 shift matrix (padded to 128 cols so FWL
    engages) handles x+4-j, the rhs AP offset (4-i) handles y+4-i.
  - cnt for all 5 blocks batches at the top of each pass: banded 2.0-matrix
    lhsT folds the j-sum (5 matmuls per PSUM tile, block dim on the rhs free
    AP); folds the final *0.5 into 1/(2 cnt).  Per-block DVE reciprocals
    trail one block behind so they never stall the multiply FIFO.
  - finals: ACT evacuates img PSUM -> one DVE bf16 2x multiply by 1/(2cnt)
    -> 3 ACT Identity activations add the per-channel mean (per-partition
    bias AP).  GpSimd is kept idle: its SBUF port is shared with the DVE
    and streaming on it slows the multiply TTs ~60%.
  - the whole pass's output stages in one [124, 5*399] tile and ships as a
    single 124-descriptor DMA (many sub-1KB descriptors double the
    semaphore-packet load on SDMA engines 0-3, the DMA bottleneck).
  - the 40-partition tail x-block sits at partition base 64 so its
    descriptors land on the underloaded odd SDMA engines.
Per-rep steady state is HBM-bound: ~12 MB/core/rep at the ~330 GB/s
effective per-core bandwidth when all 8 cores stream (plus ~6us/rep of
semaphore/refill packets pinned to SDMA engine 0).
"""

import ml_dtypes
import numpy as np

import concourse.bass as bass
import concourse.mybir as mybir
import concourse.tile as tile
from concourse import bacc
from concourse.bass_utils import run_bass_kernel_spmd

F32 = mybir.dt.float32
BF16 = mybir.dt.bfloat16
AX = mybir.AxisListType
ALU = mybir.AluOpType
ACTF = mybir.ActivationFunctionType

PS = 5
PH = PW = 536
H = W = 532
PD = 75
NBAND = 4
BAND_Y = 133          # output rows per band
BAND_R = 137          # patch rows per band (halo of PS-1)
RP = 138              # padded patch-row pitch (even -> keeps bf16 2x packing)
NPIX_CH = H * W       # 283024, per-channel pixel count
FD = 3 * BAND_Y       # 399 free elements of the img/out tiles

# x-blocks: (x0, nx, nq, pb)  with q-range [x0, x0 + nq), loaded at SBUF
# partition base pb.  The 40-partition tail block sits at pb=64: partitions
# 0-39 map to the even SDMA engines, which already carry the fullest load;
# 64-103 map to the underloaded odd engines, leveling the DMA stream.
XBLKS = [(0, 124, 128, 0), (124, 124, 128, 0), (248, 124, 128, 0),
         (372, 124, 128, 0), (496, 36, 40, 64)]

# For_i body unrolling for reps>1 timing programs (the loop wrap drains the
# pipeline for ~15us; 48 passes amortize it below the timing noise)
LOOP_UNROLL = 48


def _ap_p(base: bass.AP, npart: int, extra_off: int, dims):
    """Custom strided view of a tile: partition dim of `base` overridden to
    `npart`, free dims replaced."""
    part = [[base.ap[0][0], npart]]
    return bass.AP(base.tensor, base.offset + extra_off, part + [list(d) for d in dims])


def build_program(reps: int = 1, ablate: str = ""):
    """Build (and compile) the single-core Bass program. SPMD: all 8 cores run
    it on their own band slice. Returns the Bacc object."""
    nc = bacc.Bacc("TRN2", target_bir_lowering=False, debug=False,
                   enable_asserts=False, num_devices=8)

    deno_d = nc.dram_tensor("deno", [PW, PD, RP], BF16, kind="ExternalInput")
    wt_d = nc.dram_tensor("wt", [128, len(XBLKS) * RP], BF16,
                          kind="ExternalInput")
    noisy_d = nc.dram_tensor("noisy", [3, H, W], BF16, kind="ExternalInput")
    # out layout [x_local=124, (block, ch, y)]: one contiguous 4KB run per
    # partition -> the whole pass's output ships as ONE 124-descriptor DMA
    # (5 per-block DMAs = 620 sub-1KB descriptors cost ~2x the semaphore
    # packets on SDMA engines 0-3, the busiest ones).
    out_d = nc.dram_tensor("out", [124, len(XBLKS) * FD], BF16,
                           kind="ExternalOutput")

    with tile.TileContext(nc) as tc:
        with (
            tc.tile_pool(name="const", bufs=1) as const_p,
            tc.tile_pool(name="deno", bufs=7) as deno_p,
            tc.tile_pool(name="wq", bufs=2) as wq_p,
            tc.tile_pool(name="small", bufs=2) as small_p,
            tc.tile_pool(name="o1", bufs=2) as o1_p,
            tc.tile_pool(name="stage", bufs=3) as stage_p,
            tc.tile_pool(name="noisy", bufs=1) as noisy_p,
            tc.tile_pool(name="psI", bufs=3, space=bass.MemorySpace.PSUM) as psI,
            tc.tile_pool(name="psC", bufs=2, space=bass.MemorySpace.PSUM) as psC,
            tc.tile_pool(name="psW", bufs=1, space=bass.MemorySpace.PSUM) as psW,
        ):
            # ---- constants ----
            # shift identities, padded to 128 cols so FWL engages:
            # shifts[pb][j][q, m] = 1.0 iff q - pb == m + 4 - j
            def mkshift(tag, j, v, pb):
                sh = const_p.tile([128, 128], BF16, tag=tag)
                nc.gpsimd.memset(sh[:], 0.0)
                nc.gpsimd.affine_select(
                    out=sh[:], in_=sh[:], compare_op=ALU.not_equal, fill=v,
                    base=j - 4 - pb, pattern=[[-1, 128]],
                    channel_multiplier=1)
                return sh
            # banded cnt matrix: band2[pb][q, m] = 2.0 iff 0 <= q-pb-m <= 4
            # (sum over j of the 5 shift matrices, scaled by 2)
            def mkband(tag, pb):
                bd = const_p.tile([128, 128], BF16, tag=tag)
                nc.gpsimd.memset(bd[:], 0.0)
                for j in range(PS):
                    nc.gpsimd.affine_select(
                        out=bd[:], in_=bd[:], compare_op=ALU.not_equal,
                        fill=2.0, base=j - 4 - pb, pattern=[[-1, 128]],
                        channel_multiplier=1)
                return bd
            PBS = sorted({pb for (_, _, _, pb) in XBLKS})
            shifts = {pb: [mkshift(f"shift{pb}_{j}", j, 1.0, pb)
                           for j in range(PS)] for pb in PBS}
            band2 = {pb: mkband(f"band2_{pb}", pb) for pb in PBS}

            ones76 = const_p.tile([76, 1], BF16, tag="ones76")
            nc.gpsimd.memset(ones76[:], 1.0)
            onesrow = const_p.tile([1, 128], F32, tag="onesrow")
            nc.gpsimd.memset(onesrow[:], 1.0 / NPIX_CH)

            # ---- per-channel means of raw noisy ----
            sums = const_p.tile([1, 3], F32, tag="sums")
            for ch in range(3):
                npix = noisy_p.tile([76, 3724], BF16, tag="noisy")
                nc.sync.dma_start(
                    out=npix[:],
                    in_=bass.AP(noisy_d, ch * NPIX_CH, [[3724, 76], [1, 3724]]))
                msum = psW.tile([1, 512], F32, tag="psw")
                nchunk = (3724 + 511) // 512
                for ci in range(nchunk):
                    c0 = ci * 512
                    n = min(512, 3724 - c0)
                    nc.tensor.matmul(
                        out=msum[0:1, 0:n],
                        lhsT=ones76[:],
                        rhs=npix[:, c0:c0 + n],
                        start=(ci == 0), stop=(ci == nchunk - 1))
                nc.vector.tensor_reduce(
                    out=sums[0:1, ch:ch + 1], in_=msum[0:1, 0:512],
                    axis=AX.X, op=ALU.add)
            mrep_ps = psW.tile([128, 3], F32, tag="psw")
            nc.tensor.matmul(out=mrep_ps[:], lhsT=onesrow[:],
                             rhs=sums[:], start=True, stop=True)
            mean_rep = const_p.tile([128, 3], F32, tag="mean_rep")
            nc.scalar.copy(mean_rep[:], mrep_ps[:])


            # ---- main loop ----
            # reps>1 wraps the body in a For_i hardware loop (for timing runs)
            UNROLL = LOOP_UNROLL
            import contextlib
            loop_cm = (tc.For_i(0, (reps + UNROLL - 1) // UNROLL, 1,
                                staggered_reset=True)
                       if reps > 1 else contextlib.nullcontext())
            n_passes = UNROLL if reps > 1 else 1
            if "nomm" in ablate and "nofin" not in ablate:
                ablate = ablate + " nofin"
            with loop_cm:
              # finals are software-pipelined one block behind the front-end
              # so the PE-consuming ops never stall their engine queues
              # waiting on this block's matmuls.
              pend = []
              rc_pend = []
              for _pass in range(n_passes):
                # wq rides the SP HWDGE ring: on the ACT ring its issue sits
                # behind the previous pass's PSUM-evac COPY + out-DMA (which
                # wait on that pass's tail), serializing every pass boundary
                # by ~15us.  On SP it issues as soon as the previous slab
                # D2Ds have, draining during the previous pass's compute.
                wq = wq_p.tile([128, len(XBLKS) * RP], BF16, tag="wq")
                nc.sync.dma_start(out=wq[:], in_=wt_d[:, :])

                # cnt for ALL blocks (depends only on wq): blocks 0-2 in one
                # PSUM tile, 3-4 in another.  Banded lhsT folds the j-sum;
                # the i-shift is the rhs AP offset; the block dim rides the
                # rhs free AP (the band matrix is block-local in q).  Emitted
                # as a closure so the matmuls land in the PE FIFO after block
                # 0's img matmuls (never delaying them), while the per-block
                # reciprocals trail one block behind in the DVE FIFO.
                cntA = psC.tile([128, 3 * BAND_Y], F32, tag="cntA")
                cntB = psC.tile([128, 2 * BAND_Y], F32, tag="cntB")
                rcA = small_p.tile([124, 3 * BAND_Y], F32, tag="rcA")
                rcB = small_p.tile([124, 2 * BAND_Y], F32, tag="rcB")
                # bf16 copies (pitch 134 keeps per-block slices 4B-aligned
                # for the finals' DVE 2x packed mode)
                rcAb = small_p.tile([124, 3 * 134], BF16, tag="rcAb")
                rcBb = small_p.tile([124, 2 * 134], BF16, tag="rcBb")
                # whole-pass output staging tile (one 124-descriptor DMA at
                # pass end).  The block-4 tail rows are never computed; zero
                # them so the (ignored-by-host) DMA bytes are finite.
                st_all = stage_p.tile([124, len(XBLKS) * FD], BF16, tag="st")
                nc.gpsimd.memset(st_all[0:124, 4 * FD:5 * FD], 0.0)

                def cnt_mms(cntA=cntA, cntB=cntB):
                    WQS = len(XBLKS) * RP
                    for i in range(PS):
                        nc.tensor.matmul(
                            out=cntA[:, :],
                            lhsT=band2[0][:, :],
                            rhs=_ap_p(wq[:], 128, (4 - i),
                                      [[RP, 3], [1, BAND_Y]]),
                            start=(i == 0), stop=(i == PS - 1))
                    # block 3 (pb=0) opens the cntB bank; block 4 (pb=64)
                    # writes the disjoint second slice (has_written=0 there
                    # after the group start -> first write overwrites).
                    for i in range(PS):
                        nc.tensor.matmul(
                            out=cntB[:, 0:BAND_Y],
                            lhsT=band2[0][:, :],
                            rhs=_ap_p(wq[:], 128, 3 * RP + (4 - i),
                                      [[1, BAND_Y]]),
                            start=(i == 0), stop=False)
                    x0, nx, nq, pb = XBLKS[4]
                    for i in range(PS):
                        nc.tensor.matmul(
                            out=cntB[:, BAND_Y:2 * BAND_Y],
                            lhsT=band2[pb][pb:pb + nq, :],
                            rhs=_ap_p(wq[:], nq,
                                      pb * WQS + 4 * RP + (4 - i),
                                      [[1, BAND_Y]]),
                            start=False, stop=(i == PS - 1))

                def mk_rc(b, cntA=cntA, cntB=cntB, rcA=rcA, rcB=rcB,
                          rcAb=rcAb, rcBb=rcBb):
                    # per-block reciprocal slice (block 4 only has 36 cols;
                    # the rest of its cnt slice is 0 -> skip, 1/0 is inf),
                    # then a bf16 downconvert for the finals 2x TT
                    src, dst, dstb, off = ((cntA, rcA, rcAb, b) if b < 3 else
                                           (cntB, rcB, rcBb, b - 3))
                    np_ = 124 if b < 4 else 36
                    def rc():
                        nc.vector.reciprocal_approx_fast(
                            dst[0:np_, off * BAND_Y:(off + 1) * BAND_Y],
                            src[0:np_, off * BAND_Y:(off + 1) * BAND_Y])
                        nc.vector.tensor_copy(
                            out=dstb[0:np_, off * 134:off * 134 + BAND_Y],
                            in_=dst[0:np_, off * BAND_Y:(off + 1) * BAND_Y])
                    return rc

                for b, (x0, nx, nq, pb) in enumerate(XBLKS):
                    WQS = len(XBLKS) * RP
                    DTS = PD * RP
                    dt = deno_p.tile([128, PD * RP], BF16, tag="deno")
                    # the [q, d, r] slab loads in 5 j-chunk DMAs (the host
                    # orders d as (j, i, ch), so each chunk is contiguous
                    # 4.1KB per partition): the DVE multiply for chunk j
                    # starts as soon as its fifth of the slab lands instead
                    # of waiting for the whole 20.7KB.
                    if "nodma" not in ablate:
                        # 2 DMAs per slab (j-chunks {0,1} and {2,3,4}): a
                        # compromise between early DVE start (subtile deps
                        # let TT_0 go after the first 2/5 of the slab) and
                        # DMA count — every DMA costs ~16 semaphore packets
                        # that all land on SDMA engines 0-3.
                        for (j0, j1) in ((0, 2), (2, PS)):
                            nc.sync.dma_start(
                                out=dt[pb:pb + nq,
                                       j0 * 15 * RP:j1 * 15 * RP],
                                in_=bass.AP(deno_d,
                                            x0 * PD * RP + j0 * 15 * RP,
                                            [[PD * RP, nq],
                                             [1, (j1 - j0) * 15 * RP]]))
                    img = None
                    if "nomm" not in ablate:
                        img = psI.tile([128, FD], F32, tag="img")
                    # wd = deno * w in 5 j-chunks (d-rows (j,i,ch)); each
                    # chunk immediately feeds its 5 img taps so the PE wakes
                    # every ~1.2us and the HAM clock gate stays at full rate.
                    for j in range(PS):
                        if "nott" not in ablate:
                            nc.vector.tensor_tensor(
                                out=_ap_p(dt[:], nq, pb * DTS + j * 15 * RP,
                                          [[RP, 15], [1, RP]]),
                                in0=_ap_p(dt[:], nq, pb * DTS + j * 15 * RP,
                                          [[RP, 15], [1, RP]]),
                                in1=_ap_p(wq[:], nq, pb * WQS + b * RP,
                                          [[0, 15], [1, RP]]),
                                op=ALU.mult)
                        # img[x, (ch,y)] accumulates the 5 taps of this j:
                        # tap (i,j): rhs = wd[q, d=(j,i,ch), r=y+4-i]
                        if "nomm" not in ablate:
                            for i in range(PS):
                                nc.tensor.matmul(
                                    out=img[:, :],
                                    lhsT=shifts[pb][j][pb:pb + nq, :],
                                    rhs=_ap_p(dt[:], nq,
                                              pb * DTS +
                                              (j * 15 + i * 3) * RP + (4 - i),
                                              [[RP, 3], [1, BAND_Y]]),
                                    start=(j == 0 and i == 0),
                                    stop=(j == PS - 1 and i == PS - 1))
                    if b == 0 and "nomm" not in ablate:
                        cnt_mms()

                    # finals (deferred 1 block): ACT evacuates img PSUM, one
                    # DVE bf16 2x multiply applies 1/(2cnt), then 3 ACT
                    # Identity activations add the per-channel mean (bias is
                    # a per-partition AP, so ACT can do it — keeping both
                    # the mean matmuls off the PE and the add off DVE and
                    # GpSimd, whose SBUF port contends with the DVE).
                    def finals(b=b, x0=x0, nx=nx, img=img, rcAb=rcAb,
                               rcBb=rcBb, st_all=st_all):
                        if "nofin" not in ablate:
                            rc_t, off = (rcAb, b) if b < 3 else (rcBb, b - 3)
                            o1 = o1_p.tile([124, FD], BF16, tag="o1")
                            tmp = stage_p.tile([124, FD], BF16, tag="tmp")
                            nc.scalar.copy(o1[0:nx, :], img[0:nx, :])
                            nc.vector.tensor_tensor(
                                out=tmp[0:nx, :],
                                in0=o1[0:nx, :],
                                in1=_ap_p(rc_t[:], nx, off * 134,
                                          [[0, 3], [1, BAND_Y]]),
                                op=ALU.mult)
                            for ch in range(3):
                                nc.scalar.activation(
                                    out=st_all[0:nx,
                                               b * FD + ch * BAND_Y:
                                               b * FD + (ch + 1) * BAND_Y],
                                    in_=tmp[0:nx,
                                            ch * BAND_Y:(ch + 1) * BAND_Y],
                                    func=ACTF.Identity,
                                    bias=mean_rep[0:nx, ch:ch + 1])
                        else:
                            nc.gpsimd.memset(
                                st_all[:, b * FD:(b + 1) * FD], 0.0)
                        if b == len(XBLKS) - 1 and "noout" not in ablate:
                            nc.scalar.dma_start(
                                out=bass.AP(out_d, 0,
                                            [[len(XBLKS) * FD, 124],
                                             [1, len(XBLKS) * FD]]),
                                in_=st_all[0:124, :])
                    if "nomm" not in ablate and "nofin" not in ablate:
                        rc_pend.append(mk_rc(b))
                        if len(rc_pend) > 1:
                            rc_pend.pop(0)()
                    pend.append(finals)
                    if len(pend) > 1:
                        pend.pop(0)()
              for rc in rc_pend:
                  rc()
              rc_pend.clear()
              for f in pend:
                  f()
              pend.clear()

    nc.compile()
    return nc


_CACHE = {}


def _get_program(reps: int = 1, ablate: str = ""):
    key = (reps, ablate)
    if key not in _CACHE:
        _CACHE[key] = build_program(reps, ablate)
    return _CACHE[key]


_DPERM = np.array([ch * 25 + i * 5 + j
                   for j in range(PS) for i in range(PS) for ch in range(3)])


def make_in_maps(noisy, deno, patch_weights):
    in_maps = []
    bf = ml_dtypes.bfloat16
    for core in range(8):
        t, b = divmod(core, NBAND)
        dband = deno[t].reshape(PH, PW, PD)[133 * b:133 * b + BAND_R]
        dband = dband.transpose(1, 2, 0)[:, _DPERM]  # [q=536, d'=(j,i,ch), r=137]
        dpad = np.zeros((PW, PD, RP), dtype=bf)
        dpad[:, :, :BAND_R] = dband.astype(bf)
        wband = patch_weights[t, :, 0].reshape(PH, PW)[133 * b:133 * b + BAND_R]
        wband = wband.T                            # [q=536, r=137]
        wtile = np.zeros((128, len(XBLKS) * RP), dtype=bf)
        for blk, (x0, nx, nq, pb) in enumerate(XBLKS):
            wtile[pb:pb + nq, blk * RP:blk * RP + BAND_R] = \
                wband[x0:x0 + nq].astype(bf)
        in_maps.append({
            "deno": dpad,
            "wt": wtile,
            "noisy": np.ascontiguousarray(noisy[t]).astype(bf),
        })
    return in_maps


def unpack_out(arr):
    """Device out [124, (block, ch, y)] bf16 -> [3, 133, 532] f32."""
    a = np.asarray(arr).astype(np.float32).reshape(124, len(XBLKS), 3, BAND_Y)
    x = np.empty((W, 3, BAND_Y), np.float32)
    for b, (x0, nx, nq, pb) in enumerate(XBLKS):
        x[x0:x0 + nx] = a[0:nx, b]
    return x.transpose(1, 2, 0)


def assemble(results):
    out = np.empty((2, 3, H, W), dtype=np.float32)
    for core in range(8):
        t, b = divmod(core, NBAND)
        out[t, :, 133 * b:133 * b + BAND_Y, :] = unpack_out(results[core]["out"])
    return out


def kernel(noisy, deno, patch_weights, inds=None, pixels_h=None, pixels_w=None,
           patches_h=None, patches_w=None, **_):
    noisy = np.asarray(noisy, dtype=np.float32)
    deno = np.asarray(deno, dtype=np.float32)
    patch_weights = np.asarray(patch_weights, dtype=np.float32)
    nc = _get_program()
    res = run_bass_kernel_spmd(nc, make_in_maps(noisy, deno, patch_weights),
                               core_ids=list(range(8)))
    return assemble(res.results)


# revision 49
# speedup vs baseline: 1.0234x; 1.0088x over previous
"""Trainium2 Bass kernel for BaseLIDIA weighted overlap-add (fold) network.

Math (derived from the reference):
  out[t,ch,y,x] = 0.5 * img[t,ch,y,x] / cnt[t,y,x] + mean(noisy[t,ch])
  img[ch,y,x]   = sum_{i,j in 0..4} deno[t, (y+4-i)*536 + (x+4-j), ch*25+i*5+j]
                                    * w[t, (y+4-i)*536 + (x+4-j)]
  cnt[y,x]      = sum_{i,j in 0..4} w[t, (y+4-i)*536 + (x+4-j)]
(`inds` is unused by the reference; the pre/post scaling collapses so that the
only use of `noisy` is its raw per-channel mean.)

Sharding: 8 cores = 2 frames x 4 row-bands of 133 output rows. Each core gets
patch rows [133b, 133b+137) (4-row halo) of its frame.

Per-core on-device algorithm (patch columns q on SBUF partitions, host layout
[q, d'=(j,i,ch), r] with r padded to 138 so every engine AP is unit-stride
innermost; d reordered on the host so each j-group of taps is contiguous):
  - deno band slab [q<=128, 75, 138] bf16 loads in 2 chunk DMAs per x-block
    on the SP HWDGE ring (w rides the same ring, prefetched a pass ahead —
    on the ACT ring its issue order serialized every pass boundary).
  - wd = deno * w in 5 j-chunk DVE tensor_tensors (w broadcast over d as the
    OUTER free dim; unit-stride inner -> 2x_1P packed mode).  Each chunk
    immediately feeds its 5 img taps so the PE never idles long enough for
    the HAM clock gate to re-throttle it to 1.2 GHz.
  - img[x, (ch,y)] = PSUM accumulation of 25 shifted matmuls (one per fold
    tap (i,j)): stationary 0/1 shift matrix (padded to 128 cols so FWL
    engages) handles x+4-j, the rhs AP offset (4-i) handles y+4-i.
  - cnt for all 5 blocks batches at the top of each pass: banded 2.0-matrix
    lhsT folds the j-sum (5 matmuls per PSUM tile, block dim on the rhs free
    AP); folds the final *0.5 into 1/(2 cnt).  Per-block DVE reciprocals
    trail one block behind so they never stall the multiply FIFO.
  - finals: ACT evacuates img PSUM -> one DVE bf16 2x multiply by 1/(2cnt)
    -> 3 ACT Identity activations add the per-channel mean (per-partition
    bias AP).  GpSimd is kept idle: its SBUF port is shared with the DVE
    and streaming on it slows the multiply TTs ~60%.
  - the whole pass's output stages in one [124, 5*399] tile and ships as a
    single 124-descriptor DMA (many sub-1KB descriptors double the
    semaphore-packet load on SDMA engines 0-3, the DMA bottleneck).
  - the 40-partition tail x-block sits at partition base 64 so its
    descriptors land on the underloaded odd SDMA engines.
Per-rep steady state is HBM-bound: ~12 MB/core/rep at the ~330 GB/s
effective per-core bandwidth when all 8 cores stream (plus ~6us/rep of
semaphore/refill packets pinned to SDMA engine 0).
"""

import ml_dtypes
import numpy as np

import concourse.bass as bass
import concourse.mybir as mybir
import concourse.tile as tile
from concourse import bacc
from concourse.bass_utils import run_bass_kernel_spmd

F32 = mybir.dt.float32
BF16 = mybir.dt.bfloat16
AX = mybir.AxisListType
ALU = mybir.AluOpType
ACTF = mybir.ActivationFunctionType

PS = 5
PH = PW = 536
H = W = 532
PD = 75
NBAND = 4
BAND_Y = 133          # output rows per band
BAND_R = 137          # patch rows per band (halo of PS-1)
RP = 138              # padded patch-row pitch (even -> keeps bf16 2x packing)
NPIX_CH = H * W       # 283024, per-channel pixel count
FD = 3 * BAND_Y       # 399 free elements of the img/out tiles

# x-blocks: (x0, nx, nq, pb)  with q-range [x0, x0 + nq), loaded at SBUF
# partition base pb.  The 40-partition tail block sits at pb=64: partitions
# 0-39 map to the even SDMA engines, which already carry the fullest load;
# 64-103 map to the underloaded odd engines, leveling the DMA stream.
XBLKS = [(0, 124, 128, 0), (124, 124, 128, 0), (248, 124, 128, 0),
         (372, 124, 128, 0), (496, 36, 40, 64)]

# For_i body unrolling for reps>1 timing programs (the loop wrap drains the
# pipeline for ~15us; 48 passes amortize it below the timing noise)
LOOP_UNROLL = 64


def _ap_p(base: bass.AP, npart: int, extra_off: int, dims):
    """Custom strided view of a tile: partition dim of `base` overridden to
    `npart`, free dims replaced."""
    part = [[base.ap[0][0], npart]]
    return bass.AP(base.tensor, base.offset + extra_off, part + [list(d) for d in dims])


def build_program(reps: int = 1, ablate: str = ""):
    """Build (and compile) the single-core Bass program. SPMD: all 8 cores run
    it on their own band slice. Returns the Bacc object."""
    nc = bacc.Bacc("TRN2", target_bir_lowering=False, debug=False,
                   enable_asserts=False, num_devices=8)

    deno_d = nc.dram_tensor("deno", [PW, PD, RP], BF16, kind="ExternalInput")
    wt_d = nc.dram_tensor("wt", [128, len(XBLKS) * RP], BF16,
                          kind="ExternalInput")
    noisy_d = nc.dram_tensor("noisy", [3, H, W], BF16, kind="ExternalInput")
    # out layout [x_local=124, (block, ch, y)]: one contiguous 4KB run per
    # partition -> the whole pass's output ships as ONE 124-descriptor DMA
    # (5 per-block DMAs = 620 sub-1KB descriptors cost ~2x the semaphore
    # packets on SDMA engines 0-3, the busiest ones).
    out_d = nc.dram_tensor("out", [124, len(XBLKS) * FD], BF16,
                           kind="ExternalOutput")

    with tile.TileContext(nc) as tc:
        with (
            tc.tile_pool(name="const", bufs=1) as const_p,
            tc.tile_pool(name="deno", bufs=7) as deno_p,
            tc.tile_pool(name="wq", bufs=2) as wq_p,
            tc.tile_pool(name="small", bufs=2) as small_p,
            tc.tile_pool(name="o1", bufs=2) as o1_p,
            tc.tile_pool(name="stage", bufs=3) as stage_p,
            tc.tile_pool(name="noisy", bufs=1) as noisy_p,
            tc.tile_pool(name="psI", bufs=3, space=bass.MemorySpace.PSUM) as psI,
            tc.tile_pool(name="psC", bufs=2, space=bass.MemorySpace.PSUM) as psC,
            tc.tile_pool(name="psW", bufs=1, space=bass.MemorySpace.PSUM) as psW,
        ):
            # ---- constants ----
            # shift identities, padded to 128 cols so FWL engages:
            # shifts[pb][j][q, m] = 1.0 iff q - pb == m + 4 - j
            def mkshift(tag, j, v, pb):
                sh = const_p.tile([128, 128], BF16, tag=tag)
                nc.gpsimd.memset(sh[:], 0.0)
                nc.gpsimd.affine_select(
                    out=sh[:], in_=sh[:], compare_op=ALU.not_equal, fill=v,
                    base=j - 4 - pb, pattern=[[-1, 128]],
                    channel_multiplier=1)
                return sh
            # banded cnt matrix: band2[pb][q, m] = 2.0 iff 0 <= q-pb-m <= 4
            # (sum over j of the 5 shift matrices, scaled by 2)
            def mkband(tag, pb):
                bd = const_p.tile([128, 128], BF16, tag=tag)
                nc.gpsimd.memset(bd[:], 0.0)
                for j in range(PS):
                    nc.gpsimd.affine_select(
                        out=bd[:], in_=bd[:], compare_op=ALU.not_equal,
                        fill=2.0, base=j - 4 - pb, pattern=[[-1, 128]],
                        channel_multiplier=1)
                return bd
            PBS = sorted({pb for (_, _, _, pb) in XBLKS})
            shifts = {pb: [mkshift(f"shift{pb}_{j}", j, 1.0, pb)
                           for j in range(PS)] for pb in PBS}
            band2 = {pb: mkband(f"band2_{pb}", pb) for pb in PBS}

            ones76 = const_p.tile([76, 1], BF16, tag="ones76")
            nc.gpsimd.memset(ones76[:], 1.0)
            onesrow = const_p.tile([1, 128], F32, tag="onesrow")
            nc.gpsimd.memset(onesrow[:], 1.0 / NPIX_CH)

            # ---- per-channel means of raw noisy ----
            sums = const_p.tile([1, 3], F32, tag="sums")
            for ch in range(3):
                npix = noisy_p.tile([76, 3724], BF16, tag="noisy")
                nc.sync.dma_start(
                    out=npix[:],
                    in_=bass.AP(noisy_d, ch * NPIX_CH, [[3724, 76], [1, 3724]]))
                msum = psW.tile([1, 512], F32, tag="psw")
                nchunk = (3724 + 511) // 512
                for ci in range(nchunk):
                    c0 = ci * 512
                    n = min(512, 3724 - c0)
                    nc.tensor.matmul(
                        out=msum[0:1, 0:n],
                        lhsT=ones76[:],
                        rhs=npix[:, c0:c0 + n],
                        start=(ci == 0), stop=(ci == nchunk - 1))
                nc.vector.tensor_reduce(
                    out=sums[0:1, ch:ch + 1], in_=msum[0:1, 0:512],
                    axis=AX.X, op=ALU.add)
            mrep_ps = psW.tile([128, 3], F32, tag="psw")
            nc.tensor.matmul(out=mrep_ps[:], lhsT=onesrow[:],
                             rhs=sums[:], start=True, stop=True)
            mean_rep = const_p.tile([128, 3], F32, tag="mean_rep")
            nc.scalar.copy(mean_rep[:], mrep_ps[:])


            # ---- main loop ----
            # reps>1 wraps the body in a For_i hardware loop (for timing runs)
            UNROLL = LOOP_UNROLL
            import contextlib
            loop_cm = (tc.For_i(0, (reps + UNROLL - 1) // UNROLL, 1,
                                staggered_reset=True)
                       if reps > 1 else contextlib.nullcontext())
            n_passes = UNROLL if reps > 1 else 1
            if "nomm" in ablate and "nofin" not in ablate:
                ablate = ablate + " nofin"
            with loop_cm:
              # finals are software-pipelined one block behind the front-end
              # so the PE-consuming ops never stall their engine queues
              # waiting on this block's matmuls.
              pend = []
              rc_pend = []
              for _pass in range(n_passes):
                # wq rides the SP HWDGE ring: on the ACT ring its issue sits
                # behind the previous pass's PSUM-evac COPY + out-DMA (which
                # wait on that pass's tail), serializing every pass boundary
                # by ~15us.  On SP it issues as soon as the previous slab
                # D2Ds have, draining during the previous pass's compute.
                wq = wq_p.tile([128, len(XBLKS) * RP], BF16, tag="wq")
                nc.sync.dma_start(out=wq[:], in_=wt_d[:, :])

                # cnt for ALL blocks (depends only on wq): blocks 0-2 in one
                # PSUM tile, 3-4 in another.  Banded lhsT folds the j-sum;
                # the i-shift is the rhs AP offset; the block dim rides the
                # rhs free AP (the band matrix is block-local in q).  Emitted
                # as a closure so the matmuls land in the PE FIFO after block
                # 0's img matmuls (never delaying them), while the per-block
                # reciprocals trail one block behind in the DVE FIFO.
                cntA = psC.tile([128, 3 * BAND_Y], F32, tag="cntA")
                cntB = psC.tile([128, 2 * BAND_Y], F32, tag="cntB")
                rcA = small_p.tile([124, 3 * BAND_Y], F32, tag="rcA")
                rcB = small_p.tile([124, 2 * BAND_Y], F32, tag="rcB")
                # bf16 copies (pitch 134 keeps per-block slices 4B-aligned
                # for the finals' DVE 2x packed mode)
                rcAb = small_p.tile([124, 3 * 134], BF16, tag="rcAb")
                rcBb = small_p.tile([124, 2 * 134], BF16, tag="rcBb")
                # whole-pass output staging tile (one 124-descriptor DMA at
                # pass end).  The block-4 tail rows are never computed; zero
                # them so the (ignored-by-host) DMA bytes are finite.
                st_all = stage_p.tile([124, len(XBLKS) * FD], BF16, tag="st")
                nc.gpsimd.memset(st_all[0:124, 4 * FD:5 * FD], 0.0)

                def cnt_mms(cntA=cntA, cntB=cntB):
                    WQS = len(XBLKS) * RP
                    for i in range(PS):
                        nc.tensor.matmul(
                            out=cntA[:, :],
                            lhsT=band2[0][:, :],
                            rhs=_ap_p(wq[:], 128, (4 - i),
                                      [[RP, 3], [1, BAND_Y]]),
                            start=(i == 0), stop=(i == PS - 1))
                    # block 3 (pb=0) opens the cntB bank; block 4 (pb=64)
                    # writes the disjoint second slice (has_written=0 there
                    # after the group start -> first write overwrites).
                    for i in range(PS):
                        nc.tensor.matmul(
                            out=cntB[:, 0:BAND_Y],
                            lhsT=band2[0][:, :],
                            rhs=_ap_p(wq[:], 128, 3 * RP + (4 - i),
                                      [[1, BAND_Y]]),
                            start=(i == 0), stop=False)
                    x0, nx, nq, pb = XBLKS[4]
                    for i in range(PS):
                        nc.tensor.matmul(
                            out=cntB[:, BAND_Y:2 * BAND_Y],
                            lhsT=band2[pb][pb:pb + nq, :],
                            rhs=_ap_p(wq[:], nq,
                                      pb * WQS + 4 * RP + (4 - i),
                                      [[1, BAND_Y]]),
                            start=False, stop=(i == PS - 1))

                def mk_rc(b, cntA=cntA, cntB=cntB, rcA=rcA, rcB=rcB,
                          rcAb=rcAb, rcBb=rcBb):
                    # per-block reciprocal slice (block 4 only has 36 cols;
                    # the rest of its cnt slice is 0 -> skip, 1/0 is inf),
                    # then a bf16 downconvert for the finals 2x TT
                    src, dst, dstb, off = ((cntA, rcA, rcAb, b) if b < 3 else
                                           (cntB, rcB, rcBb, b - 3))
                    np_ = 124 if b < 4 else 36
                    def rc():
                        nc.vector.reciprocal_approx_fast(
                            dst[0:np_, off * BAND_Y:(off + 1) * BAND_Y],
                            src[0:np_, off * BAND_Y:(off + 1) * BAND_Y])
                        nc.vector.tensor_copy(
                            out=dstb[0:np_, off * 134:off * 134 + BAND_Y],
                            in_=dst[0:np_, off * BAND_Y:(off + 1) * BAND_Y])
                    return rc

                for b, (x0, nx, nq, pb) in enumerate(XBLKS):
                    WQS = len(XBLKS) * RP
                    DTS = PD * RP
                    dt = deno_p.tile([128, PD * RP], BF16, tag="deno")
                    # the [q, d, r] slab loads in 5 j-chunk DMAs (the host
                    # orders d as (j, i, ch), so each chunk is contiguous
                    # 4.1KB per partition): the DVE multiply for chunk j
                    # starts as soon as its fifth of the slab lands instead
                    # of waiting for the whole 20.7KB.
                    if "nodma" not in ablate:
                        # 2 DMAs per slab (j-chunks {0,1} and {2,3,4}): a
                        # compromise between early DVE start (subtile deps
                        # let TT_0 go after the first 2/5 of the slab) and
                        # DMA count — every DMA costs ~16 semaphore packets
                        # that all land on SDMA engines 0-3.
                        for (j0, j1) in ((0, 2), (2, PS)):
                            nc.sync.dma_start(
                                out=dt[pb:pb + nq,
                                       j0 * 15 * RP:j1 * 15 * RP],
                                in_=bass.AP(deno_d,
                                            x0 * PD * RP + j0 * 15 * RP,
                                            [[PD * RP, nq],
                                             [1, (j1 - j0) * 15 * RP]]))
                    img = None
                    if "nomm" not in ablate:
                        img = psI.tile([128, FD], F32, tag="img")
                    # wd = deno * w in 5 j-chunks (d-rows (j,i,ch)); each
                    # chunk immediately feeds its 5 img taps so the PE wakes
                    # every ~1.2us and the HAM clock gate stays at full rate.
                    for j in range(PS):
                        if "nott" not in ablate:
                            nc.vector.tensor_tensor(
                                out=_ap_p(dt[:], nq, pb * DTS + j * 15 * RP,
                                          [[RP, 15], [1, RP]]),
                                in0=_ap_p(dt[:], nq, pb * DTS + j * 15 * RP,
                                          [[RP, 15], [1, RP]]),
                                in1=_ap_p(wq[:], nq, pb * WQS + b * RP,
                                          [[0, 15], [1, RP]]),
                                op=ALU.mult)
                        # img[x, (ch,y)] accumulates the 5 taps of this j:
                        # tap (i,j): rhs = wd[q, d=(j,i,ch), r=y+4-i]
                        if "nomm" not in ablate:
                            for i in range(PS):
                                nc.tensor.matmul(
                                    out=img[:, :],
                                    lhsT=shifts[pb][j][pb:pb + nq, :],
                                    rhs=_ap_p(dt[:], nq,
                                              pb * DTS +
                                              (j * 15 + i * 3) * RP + (4 - i),
                                              [[RP, 3], [1, BAND_Y]]),
                                    start=(j == 0 and i == 0),
                                    stop=(j == PS - 1 and i == PS - 1))
                    if b == 0 and "nomm" not in ablate:
                        cnt_mms()

                    # finals (deferred 1 block): ACT evacuates img PSUM, one
                    # DVE bf16 2x multiply applies 1/(2cnt), then 3 ACT
                    # Identity activations add the per-channel mean (bias is
                    # a per-partition AP, so ACT can do it — keeping both
                    # the mean matmuls off the PE and the add off DVE and
                    # GpSimd, whose SBUF port contends with the DVE).
                    def finals(b=b, x0=x0, nx=nx, img=img, rcAb=rcAb,
                               rcBb=rcBb, st_all=st_all):
                        if "nofin" not in ablate:
                            rc_t, off = (rcAb, b) if b < 3 else (rcBb, b - 3)
                            o1 = o1_p.tile([124, FD], BF16, tag="o1")
                            tmp = stage_p.tile([124, FD], BF16, tag="tmp")
                            nc.scalar.copy(o1[0:nx, :], img[0:nx, :])
                            nc.vector.tensor_tensor(
                                out=tmp[0:nx, :],
                                in0=o1[0:nx, :],
                                in1=_ap_p(rc_t[:], nx, off * 134,
                                          [[0, 3], [1, BAND_Y]]),
                                op=ALU.mult)
                            for ch in range(3):
                                nc.scalar.activation(
                                    out=st_all[0:nx,
                                               b * FD + ch * BAND_Y:
                                               b * FD + (ch + 1) * BAND_Y],
                                    in_=tmp[0:nx,
                                            ch * BAND_Y:(ch + 1) * BAND_Y],
                                    func=ACTF.Identity,
                                    bias=mean_rep[0:nx, ch:ch + 1])
                        else:
                            nc.gpsimd.memset(
                                st_all[:, b * FD:(b + 1) * FD], 0.0)
                        if b == len(XBLKS) - 1 and "noout" not in ablate:
                            nc.scalar.dma_start(
                                out=bass.AP(out_d, 0,
                                            [[len(XBLKS) * FD, 124],
                                             [1, len(XBLKS) * FD]]),
                                in_=st_all[0:124, :])
                    if "nomm" not in ablate and "nofin" not in ablate:
                        rc_pend.append(mk_rc(b))
                        if len(rc_pend) > 1:
                            rc_pend.pop(0)()
                    pend.append(finals)
                    if len(pend) > 1:
                        pend.pop(0)()
              for rc in rc_pend:
                  rc()
              rc_pend.clear()
              for f in pend:
                  f()
              pend.clear()

    nc.compile()
    return nc


_CACHE = {}


def _get_program(reps: int = 1, ablate: str = ""):
    key = (reps, ablate)
    if key not in _CACHE:
        _CACHE[key] = build_program(reps, ablate)
    return _CACHE[key]


_DPERM = np.array([ch * 25 + i * 5 + j
                   for j in range(PS) for i in range(PS) for ch in range(3)])


def make_in_maps(noisy, deno, patch_weights):
    in_maps = []
    bf = ml_dtypes.bfloat16
    for core in range(8):
        t, b = divmod(core, NBAND)
        dband = deno[t].reshape(PH, PW, PD)[133 * b:133 * b + BAND_R]
        dband = dband.transpose(1, 2, 0)[:, _DPERM]  # [q=536, d'=(j,i,ch), r=137]
        dpad = np.zeros((PW, PD, RP), dtype=bf)
        dpad[:, :, :BAND_R] = dband.astype(bf)
        wband = patch_weights[t, :, 0].reshape(PH, PW)[133 * b:133 * b + BAND_R]
        wband = wband.T                            # [q=536, r=137]
        wtile = np.zeros((128, len(XBLKS) * RP), dtype=bf)
        for blk, (x0, nx, nq, pb) in enumerate(XBLKS):
            wtile[pb:pb + nq, blk * RP:blk * RP + BAND_R] = \
                wband[x0:x0 + nq].astype(bf)
        in_maps.append({
            "deno": dpad,
            "wt": wtile,
            "noisy": np.ascontiguousarray(noisy[t]).astype(bf),
        })
    return in_maps


def unpack_out(arr):
    """Device out [124, (block, ch, y)] bf16 -> [3, 133, 532] f32."""
    a = np.asarray(arr).astype(np.float32).reshape(124, len(XBLKS), 3, BAND_Y)
    x = np.empty((W, 3, BAND_Y), np.float32)
    for b, (x0, nx, nq, pb) in enumerate(XBLKS):
        x[x0:x0 + nx] = a[0:nx, b]
    return x.transpose(1, 2, 0)


def assemble(results):
    out = np.empty((2, 3, H, W), dtype=np.float32)
    for core in range(8):
        t, b = divmod(core, NBAND)
        out[t, :, 133 * b:133 * b + BAND_Y, :] = unpack_out(results[core]["out"])
    return out


def kernel(noisy, deno, patch_weights, inds=None, pixels_h=None, pixels_w=None,
           patches_h=None, patches_w=None, **_):
    noisy = np.asarray(noisy, dtype=np.float32)
    deno = np.asarray(deno, dtype=np.float32)
    patch_weights = np.asarray(patch_weights, dtype=np.float32)
    nc = _get_program()
    res = run_bass_kernel_spmd(nc, make_in_maps(noisy, deno, patch_weights),
                               core_ids=list(range(8)))
    return assemble(res.results)


# revision 50
# speedup vs baseline: 1.0292x; 1.0057x over previous
"""Trainium2 Bass kernel for BaseLIDIA weighted overlap-add (fold) network.

Math (derived from the reference):
  out[t,ch,y,x] = 0.5 * img[t,ch,y,x] / cnt[t,y,x] + mean(noisy[t,ch])
  img[ch,y,x]   = sum_{i,j in 0..4} deno[t, (y+4-i)*536 + (x+4-j), ch*25+i*5+j]
                                    * w[t, (y+4-i)*536 + (x+4-j)]
  cnt[y,x]      = sum_{i,j in 0..4} w[t, (y+4-i)*536 + (x+4-j)]
(`inds` is unused by the reference; the pre/post scaling collapses so that the
only use of `noisy` is its raw per-channel mean.)

Sharding: 8 cores = 2 frames x 4 row-bands of 133 output rows. Each core gets
patch rows [133b, 133b+137) (4-row halo) of its frame.

Per-core on-device algorithm (patch columns q on SBUF partitions, host layout
[q, d'=(j,i,ch), r] with r padded to 138 so every engine AP is unit-stride
innermost; d reordered on the host so each j-group of taps is contiguous):
  - deno band slab [q<=128, 75, 138] bf16 loads in 2 chunk DMAs per x-block
    on the SP HWDGE ring (w rides the same ring, prefetched a pass ahead —
    on the ACT ring its issue order serialized every pass boundary).
  - wd = deno * w in 5 j-chunk DVE tensor_tensors (w broadcast over d as the
    OUTER free dim; unit-stride inner -> 2x_1P packed mode).  Each chunk
    immediately feeds its 5 img taps so the PE never idles long enough for
    the HAM clock gate to re-throttle it to 1.2 GHz.
  - img[x, (ch,y)] = PSUM accumulation of 25 shifted matmuls (one per fold
    tap (i,j)): stationary 0/1 shift matrix (padded to 128 cols so FWL
    engages) handles x+4-j, the rhs AP offset (4-i) handles y+4-i.
  - cnt for all 5 blocks batches at the top of each pass: banded 2.0-matrix
    lhsT folds the j-sum (5 matmuls per PSUM tile, block dim on the rhs free
    AP); folds the final *0.5 into 1/(2 cnt).  Per-block DVE reciprocals
    trail one block behind so they never stall the multiply FIFO.
  - finals: ACT evacuates img PSUM -> one DVE bf16 2x multiply by 1/(2cnt)
    -> 3 ACT Identity activations add the per-channel mean (per-partition
    bias AP).  GpSimd is kept idle: its SBUF port is shared with the DVE
    and streaming on it slows the multiply TTs ~60%.
  - the whole pass's output stages in one [124, 5*399] tile and ships as a
    single 124-descriptor DMA (many sub-1KB descriptors double the
    semaphore-packet load on SDMA engines 0-3, the DMA bottleneck).
  - the 40-partition tail x-block sits at partition base 64 so its
    descriptors land on the underloaded odd SDMA engines.
Per-rep steady state is HBM-bound: ~12 MB/core/rep at the ~330 GB/s
effective per-core bandwidth when all 8 cores stream (plus ~6us/rep of
semaphore/refill packets pinned to SDMA engine 0).
"""

import ml_dtypes
import numpy as np

import concourse.bass as bass
import concourse.mybir as mybir
import concourse.tile as tile
from concourse import bacc
from concourse.bass_utils import run_bass_kernel_spmd

F32 = mybir.dt.float32
BF16 = mybir.dt.bfloat16
AX = mybir.AxisListType
ALU = mybir.AluOpType
ACTF = mybir.ActivationFunctionType

PS = 5
PH = PW = 536
H = W = 532
PD = 75
NBAND = 4
BAND_Y = 133          # output rows per band
BAND_R = 137          # patch rows per band (halo of PS-1)
RP = 138              # padded patch-row pitch (even -> keeps bf16 2x packing)
NPIX_CH = H * W       # 283024, per-channel pixel count
FD = 3 * BAND_Y       # 399 free elements of the img/out tiles

# x-blocks: (x0, nx, nq, pb)  with q-range [x0, x0 + nq), loaded at SBUF
# partition base pb.  The 40-partition tail block sits at pb=64: partitions
# 0-39 map to the even SDMA engines, which already carry the fullest load;
# 64-103 map to the underloaded odd engines, leveling the DMA stream.
XBLKS = [(0, 124, 128, 0), (124, 124, 128, 0), (248, 124, 128, 0),
         (372, 124, 128, 0), (496, 36, 40, 64)]

# For_i body unrolling for reps>1 timing programs (the loop wrap drains the
# pipeline for ~15us; 48 passes amortize it below the timing noise)
LOOP_UNROLL = 96


def _ap_p(base: bass.AP, npart: int, extra_off: int, dims):
    """Custom strided view of a tile: partition dim of `base` overridden to
    `npart`, free dims replaced."""
    part = [[base.ap[0][0], npart]]
    return bass.AP(base.tensor, base.offset + extra_off, part + [list(d) for d in dims])


def build_program(reps: int = 1, ablate: str = ""):
    """Build (and compile) the single-core Bass program. SPMD: all 8 cores run
    it on their own band slice. Returns the Bacc object."""
    nc = bacc.Bacc("TRN2", target_bir_lowering=False, debug=False,
                   enable_asserts=False, num_devices=8)

    deno_d = nc.dram_tensor("deno", [PW, PD, RP], BF16, kind="ExternalInput")
    wt_d = nc.dram_tensor("wt", [128, len(XBLKS) * RP], BF16,
                          kind="ExternalInput")
    noisy_d = nc.dram_tensor("noisy", [3, H, W], BF16, kind="ExternalInput")
    # out layout [x_local=124, (block, ch, y)]: one contiguous 4KB run per
    # partition -> the whole pass's output ships as ONE 124-descriptor DMA
    # (5 per-block DMAs = 620 sub-1KB descriptors cost ~2x the semaphore
    # packets on SDMA engines 0-3, the busiest ones).
    out_d = nc.dram_tensor("out", [124, len(XBLKS) * FD], BF16,
                           kind="ExternalOutput")

    with tile.TileContext(nc) as tc:
        with (
            tc.tile_pool(name="const", bufs=1) as const_p,
            tc.tile_pool(name="deno", bufs=7) as deno_p,
            tc.tile_pool(name="wq", bufs=2) as wq_p,
            tc.tile_pool(name="small", bufs=2) as small_p,
            tc.tile_pool(name="o1", bufs=2) as o1_p,
            tc.tile_pool(name="stage", bufs=3) as stage_p,
            tc.tile_pool(name="noisy", bufs=1) as noisy_p,
            tc.tile_pool(name="psI", bufs=3, space=bass.MemorySpace.PSUM) as psI,
            tc.tile_pool(name="psC", bufs=2, space=bass.MemorySpace.PSUM) as psC,
            tc.tile_pool(name="psW", bufs=1, space=bass.MemorySpace.PSUM) as psW,
        ):
            # ---- constants ----
            # shift identities, padded to 128 cols so FWL engages:
            # shifts[pb][j][q, m] = 1.0 iff q - pb == m + 4 - j
            def mkshift(tag, j, v, pb):
                sh = const_p.tile([128, 128], BF16, tag=tag)
                nc.gpsimd.memset(sh[:], 0.0)
                nc.gpsimd.affine_select(
                    out=sh[:], in_=sh[:], compare_op=ALU.not_equal, fill=v,
                    base=j - 4 - pb, pattern=[[-1, 128]],
                    channel_multiplier=1)
                return sh
            # banded cnt matrix: band2[pb][q, m] = 2.0 iff 0 <= q-pb-m <= 4
            # (sum over j of the 5 shift matrices, scaled by 2)
            def mkband(tag, pb):
                bd = const_p.tile([128, 128], BF16, tag=tag)
                nc.gpsimd.memset(bd[:], 0.0)
                for j in range(PS):
                    nc.gpsimd.affine_select(
                        out=bd[:], in_=bd[:], compare_op=ALU.not_equal,
                        fill=2.0, base=j - 4 - pb, pattern=[[-1, 128]],
                        channel_multiplier=1)
                return bd
            PBS = sorted({pb for (_, _, _, pb) in XBLKS})
            shifts = {pb: [mkshift(f"shift{pb}_{j}", j, 1.0, pb)
                           for j in range(PS)] for pb in PBS}
            band2 = {pb: mkband(f"band2_{pb}", pb) for pb in PBS}

            ones76 = const_p.tile([76, 1], BF16, tag="ones76")
            nc.gpsimd.memset(ones76[:], 1.0)
            onesrow = const_p.tile([1, 128], F32, tag="onesrow")
            nc.gpsimd.memset(onesrow[:], 1.0 / NPIX_CH)

            # ---- per-channel means of raw noisy ----
            sums = const_p.tile([1, 3], F32, tag="sums")
            for ch in range(3):
                npix = noisy_p.tile([76, 3724], BF16, tag="noisy")
                nc.sync.dma_start(
                    out=npix[:],
                    in_=bass.AP(noisy_d, ch * NPIX_CH, [[3724, 76], [1, 3724]]))
                msum = psW.tile([1, 512], F32, tag="psw")
                nchunk = (3724 + 511) // 512
                for ci in range(nchunk):
                    c0 = ci * 512
                    n = min(512, 3724 - c0)
                    nc.tensor.matmul(
                        out=msum[0:1, 0:n],
                        lhsT=ones76[:],
                        rhs=npix[:, c0:c0 + n],
                        start=(ci == 0), stop=(ci == nchunk - 1))
                nc.vector.tensor_reduce(
                    out=sums[0:1, ch:ch + 1], in_=msum[0:1, 0:512],
                    axis=AX.X, op=ALU.add)
            mrep_ps = psW.tile([128, 3], F32, tag="psw")
            nc.tensor.matmul(out=mrep_ps[:], lhsT=onesrow[:],
                             rhs=sums[:], start=True, stop=True)
            mean_rep = const_p.tile([128, 3], F32, tag="mean_rep")
            nc.scalar.copy(mean_rep[:], mrep_ps[:])


            # ---- main loop ----
            # reps>1 wraps the body in a For_i hardware loop (for timing runs)
            UNROLL = LOOP_UNROLL
            import contextlib
            loop_cm = (tc.For_i(0, (reps + UNROLL - 1) // UNROLL, 1,
                                staggered_reset=True)
                       if reps > 1 else contextlib.nullcontext())
            n_passes = UNROLL if reps > 1 else 1
            if "nomm" in ablate and "nofin" not in ablate:
                ablate = ablate + " nofin"
            with loop_cm:
              # finals are software-pipelined one block behind the front-end
              # so the PE-consuming ops never stall their engine queues
              # waiting on this block's matmuls.
              pend = []
              rc_pend = []
              for _pass in range(n_passes):
                # wq rides the SP HWDGE ring: on the ACT ring its issue sits
                # behind the previous pass's PSUM-evac COPY + out-DMA (which
                # wait on that pass's tail), serializing every pass boundary
                # by ~15us.  On SP it issues as soon as the previous slab
                # D2Ds have, draining during the previous pass's compute.
                wq = wq_p.tile([128, len(XBLKS) * RP], BF16, tag="wq")
                nc.sync.dma_start(out=wq[:], in_=wt_d[:, :])

                # cnt for ALL blocks (depends only on wq): blocks 0-2 in one
                # PSUM tile, 3-4 in another.  Banded lhsT folds the j-sum;
                # the i-shift is the rhs AP offset; the block dim rides the
                # rhs free AP (the band matrix is block-local in q).  Emitted
                # as a closure so the matmuls land in the PE FIFO after block
                # 0's img matmuls (never delaying them), while the per-block
                # reciprocals trail one block behind in the DVE FIFO.
                cntA = psC.tile([128, 3 * BAND_Y], F32, tag="cntA")
                cntB = psC.tile([128, 2 * BAND_Y], F32, tag="cntB")
                rcA = small_p.tile([124, 3 * BAND_Y], F32, tag="rcA")
                rcB = small_p.tile([124, 2 * BAND_Y], F32, tag="rcB")
                # bf16 copies (pitch 134 keeps per-block slices 4B-aligned
                # for the finals' DVE 2x packed mode)
                rcAb = small_p.tile([124, 3 * 134], BF16, tag="rcAb")
                rcBb = small_p.tile([124, 2 * 134], BF16, tag="rcBb")
                # whole-pass output staging tile (one 124-descriptor DMA at
                # pass end).  The block-4 tail rows are never computed; zero
                # them so the (ignored-by-host) DMA bytes are finite.
                st_all = stage_p.tile([124, len(XBLKS) * FD], BF16, tag="st")
                nc.gpsimd.memset(st_all[0:124, 4 * FD:5 * FD], 0.0)

                def cnt_mms(cntA=cntA, cntB=cntB):
                    WQS = len(XBLKS) * RP
                    for i in range(PS):
                        nc.tensor.matmul(
                            out=cntA[:, :],
                            lhsT=band2[0][:, :],
                            rhs=_ap_p(wq[:], 128, (4 - i),
                                      [[RP, 3], [1, BAND_Y]]),
                            start=(i == 0), stop=(i == PS - 1))
                    # block 3 (pb=0) opens the cntB bank; block 4 (pb=64)
                    # writes the disjoint second slice (has_written=0 there
                    # after the group start -> first write overwrites).
                    for i in range(PS):
                        nc.tensor.matmul(
                            out=cntB[:, 0:BAND_Y],
                            lhsT=band2[0][:, :],
                            rhs=_ap_p(wq[:], 128, 3 * RP + (4 - i),
                                      [[1, BAND_Y]]),
                            start=(i == 0), stop=False)
                    x0, nx, nq, pb = XBLKS[4]
                    for i in range(PS):
                        nc.tensor.matmul(
                            out=cntB[:, BAND_Y:2 * BAND_Y],
                            lhsT=band2[pb][pb:pb + nq, :],
                            rhs=_ap_p(wq[:], nq,
                                      pb * WQS + 4 * RP + (4 - i),
                                      [[1, BAND_Y]]),
                            start=False, stop=(i == PS - 1))

                def mk_rc(b, cntA=cntA, cntB=cntB, rcA=rcA, rcB=rcB,
                          rcAb=rcAb, rcBb=rcBb):
                    # per-block reciprocal slice (block 4 only has 36 cols;
                    # the rest of its cnt slice is 0 -> skip, 1/0 is inf),
                    # then a bf16 downconvert for the finals 2x TT
                    src, dst, dstb, off = ((cntA, rcA, rcAb, b) if b < 3 else
                                           (cntB, rcB, rcBb, b - 3))
                    np_ = 124 if b < 4 else 36
                    def rc():
                        nc.vector.reciprocal_approx_fast(
                            dst[0:np_, off * BAND_Y:(off + 1) * BAND_Y],
                            src[0:np_, off * BAND_Y:(off + 1) * BAND_Y])
                        nc.vector.tensor_copy(
                            out=dstb[0:np_, off * 134:off * 134 + BAND_Y],
                            in_=dst[0:np_, off * BAND_Y:(off + 1) * BAND_Y])
                    return rc

                for b, (x0, nx, nq, pb) in enumerate(XBLKS):
                    WQS = len(XBLKS) * RP
                    DTS = PD * RP
                    dt = deno_p.tile([128, PD * RP], BF16, tag="deno")
                    # the [q, d, r] slab loads in 5 j-chunk DMAs (the host
                    # orders d as (j, i, ch), so each chunk is contiguous
                    # 4.1KB per partition): the DVE multiply for chunk j
                    # starts as soon as its fifth of the slab lands instead
                    # of waiting for the whole 20.7KB.
                    if "nodma" not in ablate:
                        # 2 DMAs per slab (j-chunks {0,1} and {2,3,4}): a
                        # compromise between early DVE start (subtile deps
                        # let TT_0 go after the first 2/5 of the slab) and
                        # DMA count — every DMA costs ~16 semaphore packets
                        # that all land on SDMA engines 0-3.
                        for (j0, j1) in ((0, 2), (2, PS)):
                            nc.sync.dma_start(
                                out=dt[pb:pb + nq,
                                       j0 * 15 * RP:j1 * 15 * RP],
                                in_=bass.AP(deno_d,
                                            x0 * PD * RP + j0 * 15 * RP,
                                            [[PD * RP, nq],
                                             [1, (j1 - j0) * 15 * RP]]))
                    img = None
                    if "nomm" not in ablate:
                        img = psI.tile([128, FD], F32, tag="img")
                    # wd = deno * w in 5 j-chunks (d-rows (j,i,ch)); each
                    # chunk immediately feeds its 5 img taps so the PE wakes
                    # every ~1.2us and the HAM clock gate stays at full rate.
                    for j in range(PS):
                        if "nott" not in ablate:
                            nc.vector.tensor_tensor(
                                out=_ap_p(dt[:], nq, pb * DTS + j * 15 * RP,
                                          [[RP, 15], [1, RP]]),
                                in0=_ap_p(dt[:], nq, pb * DTS + j * 15 * RP,
                                          [[RP, 15], [1, RP]]),
                                in1=_ap_p(wq[:], nq, pb * WQS + b * RP,
                                          [[0, 15], [1, RP]]),
                                op=ALU.mult)
                        # img[x, (ch,y)] accumulates the 5 taps of this j:
                        # tap (i,j): rhs = wd[q, d=(j,i,ch), r=y+4-i]
                        if "nomm" not in ablate:
                            for i in range(PS):
                                nc.tensor.matmul(
                                    out=img[:, :],
                                    lhsT=shifts[pb][j][pb:pb + nq, :],
                                    rhs=_ap_p(dt[:], nq,
                                              pb * DTS +
                                              (j * 15 + i * 3) * RP + (4 - i),
                                              [[RP, 3], [1, BAND_Y]]),
                                    start=(j == 0 and i == 0),
                                    stop=(j == PS - 1 and i == PS - 1))
                    if b == 0 and "nomm" not in ablate:
                        cnt_mms()

                    # finals (deferred 1 block): ACT evacuates img PSUM, one
                    # DVE bf16 2x multiply applies 1/(2cnt), then 3 ACT
                    # Identity activations add the per-channel mean (bias is
                    # a per-partition AP, so ACT can do it — keeping both
                    # the mean matmuls off the PE and the add off DVE and
                    # GpSimd, whose SBUF port contends with the DVE).
                    def finals(b=b, x0=x0, nx=nx, img=img, rcAb=rcAb,
                               rcBb=rcBb, st_all=st_all):
                        if "nofin" not in ablate:
                            rc_t, off = (rcAb, b) if b < 3 else (rcBb, b - 3)
                            o1 = o1_p.tile([124, FD], BF16, tag="o1")
                            tmp = stage_p.tile([124, FD], BF16, tag="tmp")
                            nc.scalar.copy(o1[0:nx, :], img[0:nx, :])
                            nc.vector.tensor_tensor(
                                out=tmp[0:nx, :],
                                in0=o1[0:nx, :],
                                in1=_ap_p(rc_t[:], nx, off * 134,
                                          [[0, 3], [1, BAND_Y]]),
                                op=ALU.mult)
                            for ch in range(3):
                                nc.scalar.activation(
                                    out=st_all[0:nx,
                                               b * FD + ch * BAND_Y:
                                               b * FD + (ch + 1) * BAND_Y],
                                    in_=tmp[0:nx,
                                            ch * BAND_Y:(ch + 1) * BAND_Y],
                                    func=ACTF.Identity,
                                    bias=mean_rep[0:nx, ch:ch + 1])
                        else:
                            nc.gpsimd.memset(
                                st_all[:, b * FD:(b + 1) * FD], 0.0)
                        if b == len(XBLKS) - 1 and "noout" not in ablate:
                            nc.scalar.dma_start(
                                out=bass.AP(out_d, 0,
                                            [[len(XBLKS) * FD, 124],
                                             [1, len(XBLKS) * FD]]),
                                in_=st_all[0:124, :])
                    if "nomm" not in ablate and "nofin" not in ablate:
                        rc_pend.append(mk_rc(b))
                        if len(rc_pend) > 1:
                            rc_pend.pop(0)()
                    pend.append(finals)
                    if len(pend) > 1:
                        pend.pop(0)()
              for rc in rc_pend:
                  rc()
              rc_pend.clear()
              for f in pend:
                  f()
              pend.clear()

    nc.compile()
    return nc


_CACHE = {}


def _get_program(reps: int = 1, ablate: str = ""):
    key = (reps, ablate)
    if key not in _CACHE:
        _CACHE[key] = build_program(reps, ablate)
    return _CACHE[key]


_DPERM = np.array([ch * 25 + i * 5 + j
                   for j in range(PS) for i in range(PS) for ch in range(3)])


def make_in_maps(noisy, deno, patch_weights):
    in_maps = []
    bf = ml_dtypes.bfloat16
    for core in range(8):
        t, b = divmod(core, NBAND)
        dband = deno[t].reshape(PH, PW, PD)[133 * b:133 * b + BAND_R]
        dband = dband.transpose(1, 2, 0)[:, _DPERM]  # [q=536, d'=(j,i,ch), r=137]
        dpad = np.zeros((PW, PD, RP), dtype=bf)
        dpad[:, :, :BAND_R] = dband.astype(bf)
        wband = patch_weights[t, :, 0].reshape(PH, PW)[133 * b:133 * b + BAND_R]
        wband = wband.T                            # [q=536, r=137]
        wtile = np.zeros((128, len(XBLKS) * RP), dtype=bf)
        for blk, (x0, nx, nq, pb) in enumerate(XBLKS):
            wtile[pb:pb + nq, blk * RP:blk * RP + BAND_R] = \
                wband[x0:x0 + nq].astype(bf)
        in_maps.append({
            "deno": dpad,
            "wt": wtile,
            "noisy": np.ascontiguousarray(noisy[t]).astype(bf),
        })
    return in_maps


def unpack_out(arr):
    """Device out [124, (block, ch, y)] bf16 -> [3, 133, 532] f32."""
    a = np.asarray(arr).astype(np.float32).reshape(124, len(XBLKS), 3, BAND_Y)
    x = np.empty((W, 3, BAND_Y), np.float32)
    for b, (x0, nx, nq, pb) in enumerate(XBLKS):
        x[x0:x0 + nx] = a[0:nx, b]
    return x.transpose(1, 2, 0)


def assemble(results):
    out = np.empty((2, 3, H, W), dtype=np.float32)
    for core in range(8):
        t, b = divmod(core, NBAND)
        out[t, :, 133 * b:133 * b + BAND_Y, :] = unpack_out(results[core]["out"])
    return out


def kernel(noisy, deno, patch_weights, inds=None, pixels_h=None, pixels_w=None,
           patches_h=None, patches_w=None, **_):
    noisy = np.asarray(noisy, dtype=np.float32)
    deno = np.asarray(deno, dtype=np.float32)
    patch_weights = np.asarray(patch_weights, dtype=np.float32)
    nc = _get_program()
    res = run_bass_kernel_spmd(nc, make_in_maps(noisy, deno, patch_weights),
                               core_ids=list(range(8)))
    return assemble(res.results)
